# revision 37
# baseline (speedup 1.0000x reference)
"""NTXEnt (intra-sample) loss kernel for Trainium2, 8-core data-parallel.

Math (matches the jax reference):
  inp [C=8, V=2, B=4096, D=512] fp32
  xn = inp / max(||inp||_D, 1e-12)
  sim[i,b,jv] = <xn[i,0,b], xn[jv,b]> / T          (T = 0.1)
  loss[i,b]   = log(exp(pos) + sum_{j!=i,v} exp(sim)) - pos
    where pos = sim[i,b,(i,1)].
  answer = mean over (i, b).

Sharding: pure data parallel over B (4096 -> 8 cores x 512).

Per-core pipeline (4 chunks of P=128 batch rows):
  1. DMA X [128b, 16iv, 512d] fp32 (b on partitions).
  2. norms^2 via 16 square+accumulate passes, split across Scalar/Vector/
     GpSimd; r = rsqrt(clamp(nn)) via exp(-0.5*ln).
  3. normalize+cast: Xn_bf16[:,iv,:] = X[:,iv,:] * r[:,iv] (one pass, split
     across the three elementwise engines).
  4. PE transpose (64 [128,128] bf16 blocks) to d-major; DVE copies
     PSUM->SBUF (2x mode) into Xd[dsub, dc, (g, iv, s)].
  5. per 16-sample group g: 4 accumulating bf16 matmuls
        W = anchors  [dsub, (i8, s16)=128]   (iv even, stride-2 view)
        M = all      [dsub, (iv16, s16)=256]
     -> G[(i,s), (iv,s')] in PSUM: all intra-group sims; cross-sample
     entries are garbage discarded by masks.
  6. E = exp(10*G) on Scalar; masked STT reductions:
        Dsum[p] = sum E*maskD   (maskD: s'==s and iv != 2i)
        pos[p]  = sum 10*G*maskP (maskP: s'==s and iv == 2i+1)
     loss = ln(Dsum) - pos.
  Host sums the [128, 32] per-core outputs and divides by C*B.
"""

import os
import numpy as np

C, V, B, D = 8, 2, 4096, 512
NCORES = 8
B_LOC = B // NCORES            # 512
P = 128                        # partitions per chunk
NCHUNK = B_LOC // P            # 4
NG = P // 16                   # 8 groups of 16 samples per chunk
TEMP_INV = 10.0                # 1 / temperature
EPS2 = 1e-24                   # (1e-12)^2 clamp on ||x||^2

# engine split for the 32 full-data passes (norms 16 + cast 16).
# GpSimd (Pool) supports neither accumulating ops nor PSUM reads, so norms
# go on act/vec only; casts run mostly as 4-iv batched tensor_tensor ops
# on pool, the rest per-iv on act (Copy w/ scale) and vec (tensor_scalar).
# Per-iv engine strings, length 16.
_NORM_ENG = os.environ.get("NTX_NORM_ENG", "aaav" "aaav" "aavv" "aaav")
_CAST_ENG = os.environ.get("NTX_CAST_ENG", "pppp" "pppp" "pppp" "aaaa")


def _host_consts():
    """maskD/maskP [128, 256] bf16, identity [128,128] bf16.

    Column order of the Gram output is (v, j, s'): col = v*128 + j*16 + s'.
    Partition order is (i, s): p = i*16 + s.
    """
    import ml_dtypes
    p_i = np.arange(128) // 16          # anchor index i of partition
    p_s = np.arange(128) % 16           # sample-in-group s of partition
    c_v = np.arange(256) // 128         # view v of column
    c_j = (np.arange(256) // 16) % 8    # crop j of column
    c_s = np.arange(256) % 16           # sample-in-group s' of column
    same = p_s[:, None] == c_s[None, :]
    self_v0 = (c_j[None, :] == p_i[:, None]) & (c_v[None, :] == 0)
    maskD = (same & ~self_v0).astype(np.float32)
    maskP = (same & (c_j[None, :] == p_i[:, None])
             & (c_v[None, :] == 1)).astype(np.float32)
    ident = np.eye(128, dtype=np.float32)
    bf16 = ml_dtypes.bfloat16
    # replicated x4 along a middle dim so the masked multiplies use plain
    # packed APs (broadcast stride-0 operands disable DVE 2x modes)
    maskD4 = np.broadcast_to(maskD[:, None, :], (P, 4, 256)).copy()
    maskP4 = np.broadcast_to(maskP[:, None, 128:], (P, 4, 128)).copy()
    return (maskD4.astype(bf16), maskP4.astype(bf16), ident.astype(bf16))


def _patch_act_tables():
    """Restrict ln/exp/square/copy/identity to the combined
    natural_log_exp_and_others set so the act-table-load pass maps every
    activation in this kernel to ONE table (walrus's per-function first
    match otherwise reloads tables on each ln<->exp switch, ~1.3us each).
    Set ids (list positions) are preserved; sets only shrink."""
    import concourse.bacc as bacc
    import concourse.mybir as mybir
    if getattr(bacc, "_ntx_act_tables_patched", False):
        return
    orig = bacc.get_activation_tables
    F = mybir.ActivationFunctionType
    ours = {F.Ln, F.Exp, F.Square, F.Copy, F.Identity}

    def patched(module_arch):
        tables = orig(module_arch)
        combined = "natural_log_exp_and_others"
        if combined in tables and ours <= tables[combined]:
            for name, funcs in tables.items():
                if name != combined:
                    tables[name] = funcs - ours
        return tables

    bacc.get_activation_tables = patched
    bacc._ntx_act_tables_patched = True


def _build_kernel():
    from contextlib import ExitStack

    import concourse.bacc as bacc
    import concourse.tile as tile
    import concourse.mybir as mybir

    _patch_act_tables()

    f32 = mybir.dt.float32
    bf16 = mybir.dt.bfloat16
    Alu = mybir.AluOpType
    Act = mybir.ActivationFunctionType

    nc = bacc.Bacc("TRN2", target_bir_lowering=False, debug=False)
    x_d = nc.dram_tensor("inp", [C, V, B_LOC, D], f32, kind="ExternalInput")
    mD_d = nc.dram_tensor("maskD", [P, 4, 256], bf16, kind="ExternalInput")
    mP_d = nc.dram_tensor("maskP", [P, 4, 128], bf16, kind="ExternalInput")
    id_d = nc.dram_tensor("ident", [P, P], bf16, kind="ExternalInput")
    o_d = nc.dram_tensor("out", [P, NCHUNK * C], f32, kind="ExternalOutput")

    with tile.TileContext(nc) as tc, ExitStack() as ctx:
        xp = ctx.enter_context(tc.tile_pool(name="x", bufs=3))
        xnp = ctx.enter_context(tc.tile_pool(name="xn", bufs=2))
        xdp = ctx.enter_context(tc.tile_pool(name="xd", bufs=2))
        sqp = ctx.enter_context(tc.tile_pool(name="sq", bufs=2))
        ep = ctx.enter_context(tc.tile_pool(name="e", bufs=3))
        scrp = ctx.enter_context(tc.tile_pool(name="scr", bufs=3))
        small = ctx.enter_context(tc.tile_pool(name="small", bufs=2))
        constp = ctx.enter_context(tc.tile_pool(name="const", bufs=1))
        outp = ctx.enter_context(tc.tile_pool(name="outp", bufs=1))
        tpp = ctx.enter_context(tc.tile_pool(name="tp", bufs=2, space="PSUM"))
        gp = ctx.enter_context(tc.tile_pool(name="g", bufs=2, space="PSUM"))

        loss_out = outp.tile([P, NCHUNK * C], f32)

        x_ap = x_d.ap()

        def issue_dma(c):
            """Load chunk c [128, 16, 512] in 4 quarter-DMAs so norms can
            start as soon as the first 4 ivs land."""
            X = xp.tile([P, C * V, D], f32)
            src = x_ap[:, :, c * P:(c + 1) * P, :].rearrange(
                "i v b d -> b (i v) d")
            for q in range(4):
                sl = slice(q * 4, (q + 1) * 4)
                nc.sync.dma_start(out=X[:, sl, :], in_=src[:, sl, :])
            return X

        # input DMAs for the first chunks go out before the constants so
        # compute can start as early as possible
        x_tiles = {0: issue_dma(0), 1: issue_dma(1)}

        # constants
        maskD = constp.tile([P, 4, 256], bf16)
        maskP = constp.tile([P, 4, P], bf16)
        ident = constp.tile([P, P], bf16)
        nc.sync.dma_start(out=maskD[:, :, :], in_=mD_d.ap())
        nc.sync.dma_start(out=maskP[:, :, :], in_=mP_d.ap())
        nc.sync.dma_start(out=ident[:, :], in_=id_d.ap())
        epsb = constp.tile([P, 1], f32)
        nc.gpsimd.memset(epsb[:, :], EPS2)

        def stage_a(c):
            """Front half of chunk c: norms, r, normalize-cast, transposes,
            PSUM->SBUF copies.  Returns the Xd tile."""
            X = x_tiles.pop(c) if c in x_tiles else issue_dma(c)

            # norms^2: nn[b, iv] = sum_d x^2  (split across act/vec)
            nn = small.tile([P, C * V], f32)
            for iv in range(C * V):
                if _NORM_ENG[iv] == "a":
                    sq = sqp.tile([P, D], bf16, tag="sq_a")
                    nc.scalar.activation(
                        out=sq[:, :], in_=X[:, iv, :], func=Act.Square,
                        accum_out=nn[:, iv:iv + 1])
                else:
                    # fp32 out: DVE's fp32->bf16 convert path is ~3x slower
                    sq = sqp.tile([P, D], f32, tag="sq_v")
                    nc.vector.scalar_tensor_tensor(
                        out=sq[:, :], in0=X[:, iv, :], scalar=1.0,
                        in1=X[:, iv, :], op0=Alu.mult, op1=Alu.mult,
                        accum_out=nn[:, iv:iv + 1])

            # r = 1/||x|| = exp(-0.5 * ln(nn + eps^2)), in two halves so
            # casts of the first 8 ivs can start early.  The eps clamp is
            # folded into Ln's bias (identical in fp32 for nn >> eps^2 and
            # for nn == 0).
            r = small.tile([P, C * V], f32)
            for h in range(2):
                sl = slice(h * C, (h + 1) * C)
                lnn = small.tile([P, C], f32, tag=f"lnn{h}")
                nc.scalar.activation(out=lnn[:, :], in_=nn[:, sl], func=Act.Ln,
                                     bias=epsb[:, 0:1])
                nc.scalar.activation(out=r[:, sl], in_=lnn[:, :], func=Act.Exp,
                                     scale=-0.5)

            # normalize + cast to bf16 (pool quarters + act/vec per-iv)
            Xn = xnp.tile([P, C * V, D], bf16)
            iv = 0
            while iv < C * V:
                eng = _CAST_ENG[iv]
                if eng == "p":
                    sl = slice(iv, iv + 4)
                    nc.gpsimd.tensor_tensor(
                        out=Xn[:, sl, :], in0=X[:, sl, :],
                        in1=r[:, sl].unsqueeze(2).broadcast_to([P, 4, D]),
                        op=Alu.mult)
                    iv += 4
                    continue
                if eng == "a":
                    nc.scalar.activation(
                        out=Xn[:, iv, :], in_=X[:, iv, :], func=Act.Copy,
                        scale=r[:, iv:iv + 1])
                else:
                    nc.vector.tensor_scalar(
                        out=Xn[:, iv, :], in0=X[:, iv, :],
                        scalar1=r[:, iv:iv + 1], scalar2=None, op0=Alu.mult)
                iv += 1

            # transpose to d-major: Xd[dsub, dc, g*256 + v*128 + i*16 + s]
            Xd = xdp.tile([P, 4, 2048], bf16)
            for dc in range(4):
                # 16 transpose blocks per dc, stored in (v, i) order so the
                # per-group anchor block (v=0) is a contiguous 128 columns
                TP = tpp.tile([P, C * V, P], bf16, tag="tp")
                for iv in range(C * V):
                    t = (iv % 2) * C + iv // 2     # t = v*8 + i
                    nc.tensor.transpose(
                        TP[:, t, :],
                        Xn[:, iv, dc * P:(dc + 1) * P],
                        ident[:, :])
                # dst col = g*256 + v*128 + i*16 + s = g*256 + t*16 + s,
                # so one strided copy covers all 16 blocks of this dc
                dstv = Xd[:, dc, :].rearrange(
                    "p (g t s) -> p t g s", g=8, t=16, s=16)
                srcv = TP[:, :, :].rearrange(
                    "p t (g s) -> p t g s", g=8, s=16)
                nc.vector.tensor_copy(out=dstv, in_=srcv)
            return Xd

        def stage_b(c, Xd):
            """Back half of chunk c: Gram matmuls, exp, masked reductions,
            loss.  Issued after stage_a(c+1) so no engine queue has chunk-c
            back-half ops blocking ready chunk-(c+1) front-half ops."""
            # sums[:, 0:8] = Dsum per group, sums[:, 8:16] = exp(pos);
            # loss = ln(Dsum) - ln(exp(pos)).  All-group exps land in one
            # E_all tile; masked reductions are batched over 4-group halves
            # (bf16 2x multiply + tensor_reduce) instead of per-group STTs.
            sums = small.tile([P, 2 * NG], f32)
            H = NG // 2
            for hh in range(2):
                # 4 groups' Grams into one 2-bank psum tile, one big exp
                Gh = gp.tile([P, H, 256], f32, tag="g")
                for lg in range(H):
                    g = hh * H + lg
                    for dc in range(4):
                        nc.tensor.matmul(
                            Gh[:, lg, :],
                            lhsT=Xd[:, dc, g * 256:g * 256 + P],
                            rhs=Xd[:, dc, g * 256:(g + 1) * 256],
                            start=(dc == 0), stop=(dc == 3))
                E = ep.tile([P, H, 256], bf16, tag="e")
                nc.scalar.activation(out=E[:, :, :], in_=Gh[:, :, :],
                                     func=Act.Exp, scale=TEMP_INV)
                # masked multiplies (plain packed bf16 APs -> DVE 2x mode),
                # reduces on vector
                scr2 = scrp.tile([P, H, 256], bf16, tag="scr2")
                nc.gpsimd.tensor_tensor(
                    out=scr2[:, :, :], in0=E[:, :, :], in1=maskD[:, :, :],
                    op=Alu.mult)
                nc.vector.tensor_reduce(
                    out=sums[:, hh * H:(hh + 1) * H], in_=scr2[:, :, :],
                    axis=mybir.AxisListType.X, op=Alu.add)
                # exp(pos) lives only in the v=1 half of the columns
                scr1 = scrp.tile([P, H, P], bf16, tag="scr1")
                nc.vector.tensor_tensor(
                    out=scr1[:, :, :], in0=E[:, :, P:256], in1=maskP[:, :, :],
                    op=Alu.mult)
                nc.vector.tensor_reduce(
                    out=sums[:, NG + hh * H:NG + (hh + 1) * H],
                    in_=scr1[:, :, :], axis=mybir.AxisListType.X, op=Alu.add)

            lnb = small.tile([P, 2 * NG], f32)
            nc.scalar.activation(out=lnb[:, :], in_=sums[:, :], func=Act.Ln)
            nc.gpsimd.tensor_tensor(
                out=loss_out[:, c * NG:(c + 1) * NG], in0=lnb[:, 0:NG],
                in1=lnb[:, NG:2 * NG], op=Alu.subtract)

        # software pipeline: A0 A1 B0 A2 B1 A3 B2 B3
        xd_tiles = {}
        xd_tiles[0] = stage_a(0)
        for c in range(1, NCHUNK):
            xd_tiles[c] = stage_a(c)
            stage_b(c - 1, xd_tiles.pop(c - 1))
        stage_b(NCHUNK - 1, xd_tiles.pop(NCHUNK - 1))

        nc.sync.dma_start(out=o_d.ap(), in_=loss_out[:, :])

    nc.compile()
    return nc


_CACHE = {}


def _get_nc():
    if "nc" not in _CACHE:
        _CACHE["nc"] = _build_kernel()
    return _CACHE["nc"]


def _run(inp, trace=False):
    from concourse.bass_utils import run_bass_kernel_spmd

    nc = _get_nc()
    maskD, maskP, ident = _host_consts()
    in_maps = []
    for k in range(NCORES):
        shard = np.ascontiguousarray(inp[:, :, k * B_LOC:(k + 1) * B_LOC, :],
                                     dtype=np.float32)
        in_maps.append({"inp": shard, "maskD": maskD, "maskP": maskP,
                        "ident": ident})
    res = run_bass_kernel_spmd(nc, in_maps, list(range(NCORES)), trace=trace)
    total = np.float64(0.0)
    for m in res.results:
        total += m["out"].astype(np.float64).sum()
    loss = np.float32(total / (C * B))
    return loss, res


def kernel(inp):
    loss, _ = _run(np.asarray(inp), trace=False)
    return loss


# revision 38
# speedup vs baseline: 1.1765x; 1.1765x over previous
"""NTXEnt (intra-sample) loss kernel for Trainium2, 8-core data-parallel.

Math (matches the jax reference):
  inp [C=8, V=2, B=4096, D=512] fp32
  xn = inp / max(||inp||_D, 1e-12)
  sim[i,b,jv] = <xn[i,0,b], xn[jv,b]> / T          (T = 0.1)
  loss[i,b]   = log(exp(pos) + sum_{j!=i,v} exp(sim)) - pos
    where pos = sim[i,b,(i,1)].
  answer = mean over (i, b).

Sharding: pure data parallel over B (4096 -> 8 cores x 512).

Per-core pipeline (4 chunks of P=128 batch rows):
  1. DMA X [128b, 16iv, 512d] fp32 (b on partitions).
  2. norms^2 via 16 square+accumulate passes, split across Scalar/Vector/
     GpSimd; r = rsqrt(clamp(nn)) via exp(-0.5*ln).
  3. normalize+cast: Xn_bf16[:,iv,:] = X[:,iv,:] * r[:,iv] (one pass, split
     across the three elementwise engines).
  4. PE transpose (64 [128,128] bf16 blocks) to d-major; DVE copies
     PSUM->SBUF (2x mode) into Xd[dsub, dc, (g, iv, s)].
  5. per 16-sample group g: 4 accumulating bf16 matmuls
        W = anchors  [dsub, (i8, s16)=128]   (iv even, stride-2 view)
        M = all      [dsub, (iv16, s16)=256]
     -> G[(i,s), (iv,s')] in PSUM: all intra-group sims; cross-sample
     entries are garbage discarded by masks.
  6. E = exp(10*G) on Scalar; masked STT reductions:
        Dsum[p] = sum E*maskD   (maskD: s'==s and iv != 2i)
        pos[p]  = sum 10*G*maskP (maskP: s'==s and iv == 2i+1)
     loss = ln(Dsum) - pos.
  Host sums the [128, 32] per-core outputs and divides by C*B.
"""

import os
import numpy as np

C, V, B, D = 8, 2, 4096, 512
NCORES = 8
B_LOC = B // NCORES            # 512
P = 128                        # partitions per chunk
NCHUNK = B_LOC // P            # 4
NG = P // 16                   # 8 groups of 16 samples per chunk
TEMP_INV = 10.0                # 1 / temperature
EPS2 = 1e-24                   # (1e-12)^2 clamp on ||x||^2

# engine split for the 32 full-data passes (norms 16 + cast 16).
# GpSimd (Pool) supports neither accumulating ops nor PSUM reads, so norms
# go on act/vec only; casts run mostly as 4-iv batched tensor_tensor ops
# on pool, the rest per-iv on act (Copy w/ scale) and vec (tensor_scalar).
# Per-iv engine strings, length 16.
_NORM_ENG = os.environ.get("NTX_NORM_ENG", "aaav" "aaav" "aavv" "aaav")
_CAST_ENG = os.environ.get("NTX_CAST_ENG", "pppp" "pppp" "pppp" "aaaa")


def _host_consts():
    """maskD/maskP [128, 256] bf16, identity [128,128] bf16.

    Column order of the Gram output is (v, j, s'): col = v*128 + j*16 + s'.
    Partition order is (i, s): p = i*16 + s.
    """
    import ml_dtypes
    p_i = np.arange(128) // 16          # anchor index i of partition
    p_s = np.arange(128) % 16           # sample-in-group s of partition
    c_v = np.arange(256) // 128         # view v of column
    c_j = (np.arange(256) // 16) % 8    # crop j of column
    c_s = np.arange(256) % 16           # sample-in-group s' of column
    same = p_s[:, None] == c_s[None, :]
    self_v0 = (c_j[None, :] == p_i[:, None]) & (c_v[None, :] == 0)
    maskD = (same & ~self_v0).astype(np.float32)
    maskP = (same & (c_j[None, :] == p_i[:, None])
             & (c_v[None, :] == 1)).astype(np.float32)
    ident = np.eye(128, dtype=np.float32)
    bf16 = ml_dtypes.bfloat16
    # replicated x4 along a middle dim so the masked multiplies use plain
    # packed APs (broadcast stride-0 operands disable DVE 2x modes)
    maskD4 = np.broadcast_to(maskD[:, None, :], (P, 4, 256)).copy()
    maskP4 = np.broadcast_to(maskP[:, None, 128:], (P, 4, 128)).copy()
    return (maskD4.astype(bf16), maskP4.astype(bf16), ident.astype(bf16))


def _patch_act_tables():
    """Restrict ln/exp/square/copy/identity to the combined
    natural_log_exp_and_others set so the act-table-load pass maps every
    activation in this kernel to ONE table (walrus's per-function first
    match otherwise reloads tables on each ln<->exp switch, ~1.3us each).
    Set ids (list positions) are preserved; sets only shrink."""
    import concourse.bacc as bacc
    import concourse.mybir as mybir
    if getattr(bacc, "_ntx_act_tables_patched", False):
        return
    orig = bacc.get_activation_tables
    F = mybir.ActivationFunctionType
    ours = {F.Ln, F.Exp, F.Square, F.Copy, F.Identity}

    def patched(module_arch):
        tables = orig(module_arch)
        combined = "natural_log_exp_and_others"
        if combined in tables and ours <= tables[combined]:
            for name, funcs in tables.items():
                if name != combined:
                    tables[name] = funcs - ours
        return tables

    bacc.get_activation_tables = patched
    bacc._ntx_act_tables_patched = True


def _build_kernel():
    from contextlib import ExitStack

    import concourse.bacc as bacc
    import concourse.tile as tile
    import concourse.mybir as mybir

    _patch_act_tables()

    f32 = mybir.dt.float32
    bf16 = mybir.dt.bfloat16
    Alu = mybir.AluOpType
    Act = mybir.ActivationFunctionType

    nc = bacc.Bacc("TRN2", target_bir_lowering=False, debug=False)
    x_d = nc.dram_tensor("inp", [C, V, B_LOC, D], f32, kind="ExternalInput")
    mD_d = nc.dram_tensor("maskD", [P, 4, 256], bf16, kind="ExternalInput")
    mP_d = nc.dram_tensor("maskP", [P, 4, 128], bf16, kind="ExternalInput")
    id_d = nc.dram_tensor("ident", [P, P], bf16, kind="ExternalInput")
    o_d = nc.dram_tensor("out", [P, NCHUNK * C], f32, kind="ExternalOutput")

    with tile.TileContext(nc) as tc, ExitStack() as ctx:
        xp = ctx.enter_context(tc.tile_pool(name="x", bufs=3))
        xnp = ctx.enter_context(tc.tile_pool(name="xn", bufs=2))
        xdp = ctx.enter_context(tc.tile_pool(name="xd", bufs=2))
        sqp = ctx.enter_context(tc.tile_pool(name="sq", bufs=2))
        ep = ctx.enter_context(tc.tile_pool(name="e", bufs=3))
        scrp = ctx.enter_context(tc.tile_pool(name="scr", bufs=3))
        small = ctx.enter_context(tc.tile_pool(name="small", bufs=2))
        constp = ctx.enter_context(tc.tile_pool(name="const", bufs=1))
        outp = ctx.enter_context(tc.tile_pool(name="outp", bufs=1))
        tpp = ctx.enter_context(tc.tile_pool(name="tp", bufs=2, space="PSUM"))
        gp = ctx.enter_context(tc.tile_pool(name="g", bufs=2, space="PSUM"))

        loss_out = outp.tile([P, NCHUNK * C], f32)

        x_ap = x_d.ap()

        def issue_dma(c):
            """Load chunk c [128, 16, 512] in 4 quarter-DMAs so norms can
            start as soon as the first 4 ivs land."""
            X = xp.tile([P, C * V, D], f32)
            src = x_ap[:, :, c * P:(c + 1) * P, :].rearrange(
                "i v b d -> b (i v) d")
            for q in range(4):
                sl = slice(q * 4, (q + 1) * 4)
                nc.sync.dma_start(out=X[:, sl, :], in_=src[:, sl, :])
            return X

        # input DMAs for the first chunks go out before the constants so
        # compute can start as early as possible
        x_tiles = {0: issue_dma(0), 1: issue_dma(1)}

        # constants
        maskD = constp.tile([P, 4, 256], bf16)
        maskP = constp.tile([P, 4, P], bf16)
        ident = constp.tile([P, P], bf16)
        nc.sync.dma_start(out=maskD[:, :, :], in_=mD_d.ap())
        nc.sync.dma_start(out=maskP[:, :, :], in_=mP_d.ap())
        nc.sync.dma_start(out=ident[:, :], in_=id_d.ap())
        epsb = constp.tile([P, 1], f32)
        nc.gpsimd.memset(epsb[:, :], EPS2)

        def stage_a(c):
            """Front half of chunk c: norms, r, normalize-cast, transposes,
            PSUM->SBUF copies.  Returns the Xd tile."""
            X = x_tiles.pop(c) if c in x_tiles else issue_dma(c)

            # norms^2: nn[b, iv] = sum_d x^2  (split across act/vec)
            nn = small.tile([P, C * V], f32)
            for iv in range(C * V):
                if _NORM_ENG[iv] == "a":
                    sq = sqp.tile([P, D], bf16, tag="sq_a")
                    nc.scalar.activation(
                        out=sq[:, :], in_=X[:, iv, :], func=Act.Square,
                        accum_out=nn[:, iv:iv + 1])
                else:
                    # fp32 out: DVE's fp32->bf16 convert path is ~3x slower
                    sq = sqp.tile([P, D], f32, tag="sq_v")
                    nc.vector.scalar_tensor_tensor(
                        out=sq[:, :], in0=X[:, iv, :], scalar=1.0,
                        in1=X[:, iv, :], op0=Alu.mult, op1=Alu.mult,
                        accum_out=nn[:, iv:iv + 1])

            # r = 1/||x|| = exp(-0.5 * ln(nn + eps^2)), in two halves so
            # casts of the first 8 ivs can start early.  The eps clamp is
            # folded into Ln's bias (identical in fp32 for nn >> eps^2 and
            # for nn == 0).
            r = small.tile([P, C * V], f32)
            for h in range(2):
                sl = slice(h * C, (h + 1) * C)
                lnn = small.tile([P, C], f32, tag=f"lnn{h}")
                nc.scalar.activation(out=lnn[:, :], in_=nn[:, sl], func=Act.Ln,
                                     bias=epsb[:, 0:1])
                nc.scalar.activation(out=r[:, sl], in_=lnn[:, :], func=Act.Exp,
                                     scale=-0.5)

            # normalize + cast to bf16 (pool quarters + act/vec per-iv)
            Xn = xnp.tile([P, C * V, D], bf16)
            iv = 0
            while iv < C * V:
                eng = _CAST_ENG[iv]
                if eng == "p":
                    sl = slice(iv, iv + 4)
                    nc.gpsimd.tensor_tensor(
                        out=Xn[:, sl, :], in0=X[:, sl, :],
                        in1=r[:, sl].unsqueeze(2).broadcast_to([P, 4, D]),
                        op=Alu.mult)
                    iv += 4
                    continue
                if eng == "a":
                    nc.scalar.activation(
                        out=Xn[:, iv, :], in_=X[:, iv, :], func=Act.Copy,
                        scale=r[:, iv:iv + 1])
                else:
                    nc.vector.tensor_scalar(
                        out=Xn[:, iv, :], in0=X[:, iv, :],
                        scalar1=r[:, iv:iv + 1], scalar2=None, op0=Alu.mult)
                iv += 1

            # transpose to d-major: Xd[dsub, dc, g*256 + v*128 + i*16 + s]
            Xd = xdp.tile([P, 4, 2048], bf16)
            for dc in range(4):
                # 16 transpose blocks per dc, stored in (v, i) order so the
                # per-group anchor block (v=0) is a contiguous 128 columns
                TP = tpp.tile([P, C * V, P], bf16, tag="tp")
                for iv in range(C * V):
                    t = (iv % 2) * C + iv // 2     # t = v*8 + i
                    nc.tensor.transpose(
                        TP[:, t, :],
                        Xn[:, iv, dc * P:(dc + 1) * P],
                        ident[:, :])
                # dst col = g*256 + v*128 + i*16 + s = g*256 + t*16 + s,
                # so one strided copy covers all 16 blocks of this dc
                dstv = Xd[:, dc, :].rearrange(
                    "p (g t s) -> p t g s", g=8, t=16, s=16)
                srcv = TP[:, :, :].rearrange(
                    "p t (g s) -> p t g s", g=8, s=16)
                nc.vector.tensor_copy(out=dstv, in_=srcv)
            return Xd

        def stage_b(c, Xd):
            """Back half of chunk c: Gram matmuls, exp, masked reductions,
            loss.  Issued after stage_a(c+1) so no engine queue has chunk-c
            back-half ops blocking ready chunk-(c+1) front-half ops."""
            # sums[:, 0:8] = Dsum per group, sums[:, 8:16] = exp(pos);
            # loss = ln(Dsum) - ln(exp(pos)).  All-group exps land in one
            # E_all tile; masked reductions are batched over 4-group halves
            # (bf16 2x multiply + tensor_reduce) instead of per-group STTs.
            sums = small.tile([P, 2 * NG], f32)
            H = NG // 2
            for hh in range(2):
                # 4 groups' Grams into one 2-bank psum tile, one big exp
                Gh = gp.tile([P, H, 256], f32, tag="g")
                for lg in range(H):
                    g = hh * H + lg
                    for dc in range(4):
                        nc.tensor.matmul(
                            Gh[:, lg, :],
                            lhsT=Xd[:, dc, g * 256:g * 256 + P],
                            rhs=Xd[:, dc, g * 256:(g + 1) * 256],
                            start=(dc == 0), stop=(dc == 3))
                E = ep.tile([P, H, 256], bf16, tag="e")
                nc.scalar.activation(out=E[:, :, :], in_=Gh[:, :, :],
                                     func=Act.Exp, scale=TEMP_INV)
                # masked multiplies (plain packed bf16 APs -> DVE 2x mode),
                # reduces on vector
                scr2 = scrp.tile([P, H, 256], bf16, tag="scr2")
                nc.vector.tensor_tensor(
                    out=scr2[:, :, :], in0=E[:, :, :], in1=maskD[:, :, :],
                    op=Alu.mult)
                nc.vector.tensor_reduce(
                    out=sums[:, hh * H:(hh + 1) * H], in_=scr2[:, :, :],
                    axis=mybir.AxisListType.X, op=Alu.add)
                # exp(pos) lives only in the v=1 half of the columns
                scr1 = scrp.tile([P, H, P], bf16, tag="scr1")
                nc.vector.tensor_tensor(
                    out=scr1[:, :, :], in0=E[:, :, P:256], in1=maskP[:, :, :],
                    op=Alu.mult)
                nc.vector.tensor_reduce(
                    out=sums[:, NG + hh * H:NG + (hh + 1) * H],
                    in_=scr1[:, :, :], axis=mybir.AxisListType.X, op=Alu.add)

            lnb = small.tile([P, 2 * NG], f32)
            nc.scalar.activation(out=lnb[:, :], in_=sums[:, :], func=Act.Ln)
            nc.gpsimd.tensor_tensor(
                out=loss_out[:, c * NG:(c + 1) * NG], in0=lnb[:, 0:NG],
                in1=lnb[:, NG:2 * NG], op=Alu.subtract)

        # software pipeline: A0 A1 B0 A2 B1 A3 B2 B3
        xd_tiles = {}
        xd_tiles[0] = stage_a(0)
        for c in range(1, NCHUNK):
            xd_tiles[c] = stage_a(c)
            stage_b(c - 1, xd_tiles.pop(c - 1))
        stage_b(NCHUNK - 1, xd_tiles.pop(NCHUNK - 1))

        nc.sync.dma_start(out=o_d.ap(), in_=loss_out[:, :])

    nc.compile()
    return nc


_CACHE = {}


def _get_nc():
    if "nc" not in _CACHE:
        _CACHE["nc"] = _build_kernel()
    return _CACHE["nc"]


def _run(inp, trace=False):
    from concourse.bass_utils import run_bass_kernel_spmd

    nc = _get_nc()
    maskD, maskP, ident = _host_consts()
    in_maps = []
    for k in range(NCORES):
        shard = np.ascontiguousarray(inp[:, :, k * B_LOC:(k + 1) * B_LOC, :],
                                     dtype=np.float32)
        in_maps.append({"inp": shard, "maskD": maskD, "maskP": maskP,
                        "ident": ident})
    res = run_bass_kernel_spmd(nc, in_maps, list(range(NCORES)), trace=trace)
    total = np.float64(0.0)
    for m in res.results:
        total += m["out"].astype(np.float64).sum()
    loss = np.float32(total / (C * B))
    return loss, res


def kernel(inp):
    loss, _ = _run(np.asarray(inp), trace=False)
    return loss


# revision 43
# speedup vs baseline: 1.2819x; 1.0896x over previous
"""NTXEnt (intra-sample) loss kernel for Trainium2, 8-core data-parallel.

Math (matches the jax reference):
  inp [C=8, V=2, B=4096, D=512] fp32
  xn = inp / max(||inp||_D, 1e-12)
  sim[i,b,jv] = <xn[i,0,b], xn[jv,b]> / T          (T = 0.1)
  loss[i,b]   = log(exp(pos) + sum_{j!=i,v} exp(sim)) - pos
    where pos = sim[i,b,(i,1)].
  answer = mean over (i, b).

Sharding: pure data parallel over B (4096 -> 8 cores x 512).

Per-core pipeline (4 chunks of P=128 batch rows):
  1. DMA X [128b, 16iv, 512d] fp32 (b on partitions).
  2. norms^2 via 16 square+accumulate passes, split across Scalar/Vector/
     GpSimd; r = rsqrt(clamp(nn)) via exp(-0.5*ln).
  3. normalize+cast: Xn_bf16[:,iv,:] = X[:,iv,:] * r[:,iv] (one pass, split
     across the three elementwise engines).
  4. PE transpose (64 [128,128] bf16 blocks) to d-major; DVE copies
     PSUM->SBUF (2x mode) into Xd[dsub, dc, (g, iv, s)].
  5. per 16-sample group g: 4 accumulating bf16 matmuls
        W = anchors  [dsub, (i8, s16)=128]   (iv even, stride-2 view)
        M = all      [dsub, (iv16, s16)=256]
     -> G[(i,s), (iv,s')] in PSUM: all intra-group sims; cross-sample
     entries are garbage discarded by masks.
  6. E = exp(10*G) on Scalar; masked STT reductions:
        Dsum[p] = sum E*maskD   (maskD: s'==s and iv != 2i)
        pos[p]  = sum 10*G*maskP (maskP: s'==s and iv == 2i+1)
     loss = ln(Dsum) - pos.
  Host sums the [128, 32] per-core outputs and divides by C*B.
"""

import os
import numpy as np

C, V, B, D = 8, 2, 4096, 512
NCORES = 8
B_LOC = B // NCORES            # 512
P = 128                        # partitions per chunk
NCHUNK = B_LOC // P            # 4
NG = P // 16                   # 8 groups of 16 samples per chunk
TEMP_INV = 10.0                # 1 / temperature
EPS2 = 1e-24                   # (1e-12)^2 clamp on ||x||^2

# engine split for the 32 full-data passes (norms 16 + cast 16).
# GpSimd (Pool) supports neither accumulating ops nor PSUM reads, so norms
# go on act/vec only; casts run mostly as 4-iv batched tensor_tensor ops
# on pool, the rest per-iv on act (Copy w/ scale) and vec (tensor_scalar).
# Per-iv engine strings, length 16.
_NORM_ENG = os.environ.get("NTX_NORM_ENG", "aaaa" "aavaaaaa" "avav")
_CAST_ENG = os.environ.get("NTX_CAST_ENG", "pppp" "pppp" "pppp" "aaaa")


def _host_consts():
    """maskD/maskP [128, 256] bf16, identity [128,128] bf16.

    Column order of the Gram output is (v, j, s'): col = v*128 + j*16 + s'.
    Partition order is (i, s): p = i*16 + s.
    """
    import ml_dtypes
    p_i = np.arange(128) // 16          # anchor index i of partition
    p_s = np.arange(128) % 16           # sample-in-group s of partition
    c_v = np.arange(256) // 128         # view v of column
    c_j = (np.arange(256) // 16) % 8    # crop j of column
    c_s = np.arange(256) % 16           # sample-in-group s' of column
    same = p_s[:, None] == c_s[None, :]
    self_v0 = (c_j[None, :] == p_i[:, None]) & (c_v[None, :] == 0)
    maskD = (same & ~self_v0).astype(np.float32)
    maskP = (same & (c_j[None, :] == p_i[:, None])
             & (c_v[None, :] == 1)).astype(np.float32)
    ident = np.eye(128, dtype=np.float32)
    bf16 = ml_dtypes.bfloat16
    # replicated x4 along a middle dim so the masked multiplies use plain
    # packed APs (broadcast stride-0 operands disable DVE 2x modes)
    maskD4 = np.broadcast_to(maskD[:, None, :], (P, 4, 256)).copy()
    maskP4 = np.broadcast_to(maskP[:, None, 128:], (P, 4, 128)).copy()
    return (maskD4.astype(bf16), maskP4.astype(bf16), ident.astype(bf16))


def _patch_act_tables():
    """Restrict ln/exp/square/copy/identity to the combined
    natural_log_exp_and_others set so the act-table-load pass maps every
    activation in this kernel to ONE table (walrus's per-function first
    match otherwise reloads tables on each ln<->exp switch, ~1.3us each).
    Set ids (list positions) are preserved; sets only shrink."""
    import concourse.bacc as bacc
    import concourse.mybir as mybir
    if getattr(bacc, "_ntx_act_tables_patched", False):
        return
    orig = bacc.get_activation_tables
    F = mybir.ActivationFunctionType
    ours = {F.Ln, F.Exp, F.Square, F.Copy, F.Identity}

    def patched(module_arch):
        tables = orig(module_arch)
        combined = "natural_log_exp_and_others"
        if combined in tables and ours <= tables[combined]:
            for name, funcs in tables.items():
                if name != combined:
                    tables[name] = funcs - ours
        return tables

    bacc.get_activation_tables = patched
    bacc._ntx_act_tables_patched = True


def _build_kernel():
    from contextlib import ExitStack

    import concourse.bacc as bacc
    import concourse.tile as tile
    import concourse.mybir as mybir

    _patch_act_tables()

    f32 = mybir.dt.float32
    bf16 = mybir.dt.bfloat16
    Alu = mybir.AluOpType
    Act = mybir.ActivationFunctionType

    nc = bacc.Bacc("TRN2", target_bir_lowering=False, debug=False)
    x_d = nc.dram_tensor("inp", [C, V, B_LOC, D], f32, kind="ExternalInput")
    mD_d = nc.dram_tensor("maskD", [P, 4, 256], bf16, kind="ExternalInput")
    mP_d = nc.dram_tensor("maskP", [P, 4, 128], bf16, kind="ExternalInput")
    id_d = nc.dram_tensor("ident", [P, P], bf16, kind="ExternalInput")
    o_d = nc.dram_tensor("out", [P, NCHUNK * C], f32, kind="ExternalOutput")

    with tile.TileContext(nc) as tc, ExitStack() as ctx:
        xp = ctx.enter_context(tc.tile_pool(name="x", bufs=3))
        xnp = ctx.enter_context(tc.tile_pool(name="xn", bufs=2))
        xdp = ctx.enter_context(tc.tile_pool(name="xd", bufs=2))
        sqp = ctx.enter_context(tc.tile_pool(name="sq", bufs=2))
        ep = ctx.enter_context(tc.tile_pool(name="e", bufs=3))
        scrp = ctx.enter_context(tc.tile_pool(name="scr", bufs=3))
        small = ctx.enter_context(tc.tile_pool(name="small", bufs=3))
        constp = ctx.enter_context(tc.tile_pool(name="const", bufs=1))
        outp = ctx.enter_context(tc.tile_pool(name="outp", bufs=1))
        tpp = ctx.enter_context(tc.tile_pool(name="tp", bufs=2, space="PSUM"))
        gp = ctx.enter_context(tc.tile_pool(name="g", bufs=2, space="PSUM"))

        loss_out = outp.tile([P, NCHUNK * C], f32)

        x_ap = x_d.ap()

        def issue_dma(c):
            """Load chunk c [128, 16, 512] in 4 quarter-DMAs so norms can
            start as soon as the first 4 ivs land."""
            X = xp.tile([P, C * V, D], f32)
            src = x_ap[:, :, c * P:(c + 1) * P, :].rearrange(
                "i v b d -> b (i v) d")
            for q in range(4):
                sl = slice(q * 4, (q + 1) * 4)
                nc.sync.dma_start(out=X[:, sl, :], in_=src[:, sl, :])
            return X

        # input DMAs for the first chunks go out before the constants so
        # compute can start as early as possible
        x_tiles = {0: issue_dma(0), 1: issue_dma(1)}

        # constants
        maskD = constp.tile([P, 4, 256], bf16)
        maskP = constp.tile([P, 4, P], bf16)
        ident = constp.tile([P, P], bf16)
        nc.sync.dma_start(out=maskD[:, :, :], in_=mD_d.ap())
        nc.sync.dma_start(out=maskP[:, :, :], in_=mP_d.ap())
        nc.sync.dma_start(out=ident[:, :], in_=id_d.ap())
        epsb = constp.tile([P, 1], f32)
        nc.gpsimd.memset(epsb[:, :], EPS2)

        def stage_a1(c):
            """DMA + norms + r for chunk c (gated only on the input DMA)."""
            X = x_tiles.pop(c) if c in x_tiles else issue_dma(c)

            # norms^2: nn[b, iv] = sum_d x^2  (split across act/vec)
            nn = small.tile([P, C * V], f32)
            for iv in range(C * V):
                if _NORM_ENG[iv] == "a":
                    sq = sqp.tile([P, D], bf16, tag="sq_a")
                    nc.scalar.activation(
                        out=sq[:, :], in_=X[:, iv, :], func=Act.Square,
                        accum_out=nn[:, iv:iv + 1])
                else:
                    # fp32 out: DVE's fp32->bf16 convert path is ~3x slower
                    sq = sqp.tile([P, D], f32, tag="sq_v")
                    nc.vector.scalar_tensor_tensor(
                        out=sq[:, :], in0=X[:, iv, :], scalar=1.0,
                        in1=X[:, iv, :], op0=Alu.mult, op1=Alu.mult,
                        accum_out=nn[:, iv:iv + 1])

            # r = 1/||x|| = exp(-0.5 * ln(nn + eps^2)), in two halves so
            # casts of the first 8 ivs can start early.  The eps clamp is
            # folded into Ln's bias (identical in fp32 for nn >> eps^2 and
            # for nn == 0).
            r = small.tile([P, C * V], f32)
            for h in range(2):
                sl = slice(h * C, (h + 1) * C)
                lnn = small.tile([P, C], f32, tag=f"lnn{h}")
                nc.scalar.activation(out=lnn[:, :], in_=nn[:, sl], func=Act.Ln,
                                     bias=epsb[:, 0:1])
                nc.scalar.activation(out=r[:, sl], in_=lnn[:, :], func=Act.Exp,
                                     scale=-0.5)
            return X, r

        def stage_a2(c, X, r):
            """Normalize-cast, transposes, PSUM->SBUF copies for chunk c
            (gated on r).  Returns the Xd tile."""
            Xn = xnp.tile([P, C * V, D], bf16)
            iv = 0
            while iv < C * V:
                eng = _CAST_ENG[iv]
                if eng == "p":
                    sl = slice(iv, iv + 4)
                    nc.gpsimd.tensor_tensor(
                        out=Xn[:, sl, :], in0=X[:, sl, :],
                        in1=r[:, sl].unsqueeze(2).broadcast_to([P, 4, D]),
                        op=Alu.mult)
                    iv += 4
                    continue
                if eng == "a":
                    nc.scalar.activation(
                        out=Xn[:, iv, :], in_=X[:, iv, :], func=Act.Copy,
                        scale=r[:, iv:iv + 1])
                else:
                    nc.vector.tensor_scalar(
                        out=Xn[:, iv, :], in0=X[:, iv, :],
                        scalar1=r[:, iv:iv + 1], scalar2=None, op0=Alu.mult)
                iv += 1

            # transpose to d-major: Xd[dsub, dc, g*256 + v*128 + i*16 + s]
            Xd = xdp.tile([P, 4, 2048], bf16)
            for dc in range(4):
                # 16 transpose blocks per dc, stored in (v, i) order so the
                # per-group anchor block (v=0) is a contiguous 128 columns
                TP = tpp.tile([P, C * V, P], bf16, tag="tp")
                for iv in range(C * V):
                    t = (iv % 2) * C + iv // 2     # t = v*8 + i
                    nc.tensor.transpose(
                        TP[:, t, :],
                        Xn[:, iv, dc * P:(dc + 1) * P],
                        ident[:, :])
                # dst col = g*256 + v*128 + i*16 + s = g*256 + t*16 + s,
                # so one strided copy covers all 16 blocks of this dc
                dstv = Xd[:, dc, :].rearrange(
                    "p (g t s) -> p t g s", g=8, t=16, s=16)
                srcv = TP[:, :, :].rearrange(
                    "p t (g s) -> p t g s", g=8, s=16)
                nc.vector.tensor_copy(out=dstv, in_=srcv)
            return Xd

        def stage_b(c, Xd):
            """Back half of chunk c: Gram matmuls, exp, masked reductions,
            loss.  Issued after stage_a(c+1) so no engine queue has chunk-c
            back-half ops blocking ready chunk-(c+1) front-half ops."""
            # sums[:, 0:8] = Dsum per group, sums[:, 8:16] = exp(pos);
            # loss = ln(Dsum) - ln(exp(pos)).  All-group exps land in one
            # E_all tile; masked reductions are batched over 4-group halves
            # (bf16 2x multiply + tensor_reduce) instead of per-group STTs.
            sums = small.tile([P, 2 * NG], f32)
            H = NG // 2
            for hh in range(2):
                # 4 groups' Grams into one 2-bank psum tile, one big exp
                Gh = gp.tile([P, H, 256], f32, tag="g")
                for lg in range(H):
                    g = hh * H + lg
                    for dc in range(4):
                        nc.tensor.matmul(
                            Gh[:, lg, :],
                            lhsT=Xd[:, dc, g * 256:g * 256 + P],
                            rhs=Xd[:, dc, g * 256:(g + 1) * 256],
                            start=(dc == 0), stop=(dc == 3))
                E = ep.tile([P, H, 256], bf16, tag="e")
                nc.scalar.activation(out=E[:, :, :], in_=Gh[:, :, :],
                                     func=Act.Exp, scale=TEMP_INV)
                # masked multiplies (plain packed bf16 APs -> DVE 2x mode),
                # reduces on vector
                scr2 = scrp.tile([P, H, 256], bf16, tag="scr2")
                nc.vector.tensor_tensor(
                    out=scr2[:, :, :], in0=E[:, :, :], in1=maskD[:, :, :],
                    op=Alu.mult)
                nc.vector.tensor_reduce(
                    out=sums[:, hh * H:(hh + 1) * H], in_=scr2[:, :, :],
                    axis=mybir.AxisListType.X, op=Alu.add)
                # exp(pos) lives only in the v=1 half of the columns
                scr1 = scrp.tile([P, H, P], bf16, tag="scr1")
                nc.vector.tensor_tensor(
                    out=scr1[:, :, :], in0=E[:, :, P:256], in1=maskP[:, :, :],
                    op=Alu.mult)
                nc.vector.tensor_reduce(
                    out=sums[:, NG + hh * H:NG + (hh + 1) * H],
                    in_=scr1[:, :, :], axis=mybir.AxisListType.X, op=Alu.add)

            lnb = small.tile([P, 2 * NG], f32)
            nc.scalar.activation(out=lnb[:, :], in_=sums[:, :], func=Act.Ln)
            nc.gpsimd.tensor_tensor(
                out=loss_out[:, c * NG:(c + 1) * NG], in0=lnb[:, 0:NG],
                in1=lnb[:, NG:2 * NG], op=Alu.subtract)

        # 3-stage software pipeline so every engine's in-order queue sees
        # ops roughly in readiness order:
        # A1(0) A1(1) A2(0) A1(2) A2(1) B(0) A1(3) A2(2) B(1) A2(3) B(2) B(3)
        xr = {0: stage_a1(0), 1: stage_a1(1)}
        xd = {0: stage_a2(0, *xr.pop(0))}
        xr[2] = stage_a1(2)
        xd[1] = stage_a2(1, *xr.pop(1))
        stage_b(0, xd.pop(0))
        xr[3] = stage_a1(3)
        xd[2] = stage_a2(2, *xr.pop(2))
        stage_b(1, xd.pop(1))
        xd[3] = stage_a2(3, *xr.pop(3))
        stage_b(2, xd.pop(2))
        stage_b(3, xd.pop(3))

        nc.sync.dma_start(out=o_d.ap(), in_=loss_out[:, :])

    nc.compile()
    return nc


_CACHE = {}


def _get_nc():
    if "nc" not in _CACHE:
        _CACHE["nc"] = _build_kernel()
    return _CACHE["nc"]


def _run(inp, trace=False):
    from concourse.bass_utils import run_bass_kernel_spmd

    nc = _get_nc()
    maskD, maskP, ident = _host_consts()
    in_maps = []
    for k in range(NCORES):
        shard = np.ascontiguousarray(inp[:, :, k * B_LOC:(k + 1) * B_LOC, :],
                                     dtype=np.float32)
        in_maps.append({"inp": shard, "maskD": maskD, "maskP": maskP,
                        "ident": ident})
    res = run_bass_kernel_spmd(nc, in_maps, list(range(NCORES)), trace=trace)
    total = np.float64(0.0)
    for m in res.results:
        total += m["out"].astype(np.float64).sum()
    loss = np.float32(total / (C * B))
    return loss, res


def kernel(inp):
    loss, _ = _run(np.asarray(inp), trace=False)
    return loss
